# revision 19
# baseline (speedup 1.0000x reference)
"""Bipartite graph convolution (cell/gene GNN message passing) on 8 Trainium2
NeuronCores.

Strategy: shard by *destination* node blocks (128 nodes per block). Each core
owns a contiguous range of cell blocks and gene blocks, so every edge's
segment-sum contribution lands on exactly one core -- no cross-core reduction.
Per destination window (128 nodes) the core:
  1. dma_gather's the source-node feature rows for all edges of the window
     (batched, one DMA per window / per 32k source chunk),
  2. builds a weighted one-hot [edge, dest_local] matrix with a single DVE
     tensor_scalar (iota == dest) * val,
  3. accumulates onehot.T @ msgs into PSUM on the TensorEngine (the segment
     sum), then computes out = relu(x_self @ W_s + bias + neighbors @ W_n)
     with the neighbor term transposed through the PE and matmul'd against
     W_n into the same PSUM accumulation.
Host work is index bucketing/padding (the sharding) and output reassembly.
"""

import sys

sys.path.insert(0, "/opt/trn_rl_repo")

import numpy as np

N_CELLS = 100000
N_GENES = 20000
N_EDGES = 640000
D = 128
P = 128
NCORES = 8
CHUNK = 32768  # int16 index range for dma_gather

# knobs / debug state
TRACE = False
LAST_RESULT = None
EDGE_BF16 = True  # bf16 edge messages/onehots (4x faster PE, ~1e-3 rel err)


def _ceil_div(a, b):
    return (a + b - 1) // b


def _core_block_ranges(n_blocks):
    """Split n_blocks into NCORES contiguous ranges as evenly as possible.
    Returns (starts, counts)."""
    base = n_blocks // NCORES
    rem = n_blocks % NCORES
    counts = np.array([base + (1 if c < rem else 0) for c in range(NCORES)])
    starts = np.concatenate([[0], np.cumsum(counts)[:-1]])
    return starts, counts


def _wrap_idx_i16(idx):
    """dma_gather index layout: index i lives at partition i%16, column i//16,
    replicated across the 8 partition groups of 16."""
    n = idx.shape[0]
    assert n % 16 == 0
    block16 = idx.reshape(n // 16, 16).T.astype(np.int16)  # [16, n/16]
    return np.tile(block16, (8, 1))  # [128, n/16]


def _prep_side(dest, src, val, n_dest, n_src):
    """Bucket edges by destination block (and source chunk), build the uniform
    per-core schedule and per-core device input arrays.

    Returns dict with:
      W: window slots per core
      sched: list of (k_chunk, T) lists per window (same for all cores)
      starts, counts: block ranges per core
      per_core: list of dicts with idx_i16 [128, 8*sumT], dest_f32 [128, sumT],
                val_f32 [128, sumT]
    """
    n_blocks = _ceil_div(n_dest, P)
    n_chunks = _ceil_div(n_src, CHUNK)
    starts, counts = _core_block_ranges(n_blocks)
    W = int(counts.max())

    block = dest // P
    chunk = src // CHUNK
    # sort edges by (block, chunk)
    order = np.lexsort((chunk, block))
    d_s = dest[order]
    s_s = src[order]
    v_s = val[order]
    key = block[order] * n_chunks + chunk[order]
    cnt = np.bincount(key, minlength=n_blocks * n_chunks).reshape(
        n_blocks, n_chunks
    )
    seg_off = np.concatenate([[0], np.cumsum(cnt.ravel())])

    # schedule: T[w][k] = max over cores of tiles needed
    T = np.zeros((W, n_chunks), dtype=np.int64)
    for c in range(NCORES):
        for w in range(int(counts[c])):
            b = int(starts[c]) + w
            T[w] = np.maximum(T[w], _ceil_div(cnt[b], P))
    # ensure at least one tile per window so every window has a defined
    # neighbor accumulation (zeros)
    for w in range(W):
        if T[w].sum() == 0:
            T[w][0] = 1

    sched = [[(k, int(T[w][k])) for k in range(n_chunks) if T[w][k] > 0]
             for w in range(W)]
    sum_T = int(T.sum())

    per_core = []
    for c in range(NCORES):
        idx_cols = []
        dest_cols = []
        val_cols = []
        for w in range(W):
            b = int(starts[c]) + w
            real = w < int(counts[c])
            for k, t in sched[w]:
                n_slots = t * P
                idx = np.zeros(n_slots, dtype=np.int64)
                dst = np.full(n_slots, -1.0, dtype=np.float32)
                vv = np.zeros(n_slots, dtype=np.float32)
                if real:
                    a = seg_off[b * n_chunks + k]
                    e = seg_off[b * n_chunks + k + 1]
                    m = e - a
                    assert m <= n_slots
                    if m > 0:
                        idx[:m] = s_s[a:e] - k * CHUNK
                        dst[:m] = (d_s[a:e] - b * P).astype(np.float32)
                        vv[:m] = v_s[a:e]
                idx_cols.append(_wrap_idx_i16(idx))
                dest_cols.append(dst.reshape(t, P).T)
                val_cols.append(vv.reshape(t, P).T)
        per_core.append({
            "idx": np.concatenate(idx_cols, axis=1),
            "dest": np.ascontiguousarray(np.concatenate(dest_cols, axis=1)),
            "val": np.ascontiguousarray(np.concatenate(val_cols, axis=1)),
        })
    return {
        "W": W,
        "sched": sched,
        "starts": starts,
        "counts": counts,
        "sum_T": sum_T,
        "n_chunks": n_chunks,
    }, per_core


def _self_slice(x, start_block, n_blocks_core, W):
    """Compact per-core self-feature tensor, BLOCK-TRANSPOSED: row (w*P + f)
    holds feature f of the window's 128 nodes, so the SBUF staging tile is
    directly usable as matmul lhsT [f_in, c]. Zero-padded."""
    out = np.zeros((W, P, D), dtype=np.float32)
    a = start_block * P
    e = min(a + n_blocks_core * P, x.shape[0])
    out.reshape(W * P, D)[: e - a] = x[a:e]
    return np.ascontiguousarray(out.transpose(0, 2, 1)).reshape(W * P, D)


def _build_nc(meta_c, meta_g, ablate=()):
    import concourse.mybir as mybir
    import concourse.tile as tile
    from concourse import bacc
    from concourse.masks import make_identity

    f32 = mybir.dt.float32
    bf16 = mybir.dt.bfloat16
    i16 = mybir.dt.int16
    i32 = mybir.dt.int32
    e_dt = bf16 if EDGE_BF16 else f32
    WC, WG = meta_c["W"], meta_g["W"]
    sTC, sTG = meta_c["sum_T"], meta_g["sum_T"]

    nc = bacc.Bacc("TRN2", target_bir_lowering=False, debug=False,
                   num_devices=NCORES)

    # DRAM tensors
    gene_x = nc.dram_tensor("gene_x", [N_GENES, D], f32, kind="ExternalInput")
    cell_x = nc.dram_tensor("cell_x", [N_CELLS, D], f32, kind="ExternalInput")
    cell_self = nc.dram_tensor("cell_self", [WC * P, D], f32,
                               kind="ExternalInput")
    gene_self = nc.dram_tensor("gene_self", [WG * P, D], f32,
                               kind="ExternalInput")
    c_idx = nc.dram_tensor("c_idx", [P, 8 * sTC], i16, kind="ExternalInput")
    c_dest = nc.dram_tensor("c_dest", [P, sTC], f32, kind="ExternalInput")
    c_val = nc.dram_tensor("c_val", [P, sTC], f32, kind="ExternalInput")
    g_idx = nc.dram_tensor("g_idx", [P, 8 * sTG], i16, kind="ExternalInput")
    g_dest = nc.dram_tensor("g_dest", [P, sTG], f32, kind="ExternalInput")
    g_val = nc.dram_tensor("g_val", [P, sTG], f32, kind="ExternalInput")
    Wcs = nc.dram_tensor("Wcs", [D, D], f32, kind="ExternalInput")
    Wcn = nc.dram_tensor("Wcn", [D, D], f32, kind="ExternalInput")
    Wgs = nc.dram_tensor("Wgs", [D, D], f32, kind="ExternalInput")
    Wgn = nc.dram_tensor("Wgn", [D, D], f32, kind="ExternalInput")
    bc = nc.dram_tensor("bc", [1, D], f32, kind="ExternalInput")
    bg = nc.dram_tensor("bg", [1, D], f32, kind="ExternalInput")
    cell_out = nc.dram_tensor("cell_out_part", [WC * P, D], f32,
                              kind="ExternalOutput")
    gene_out = nc.dram_tensor("gene_out_part", [WG * P, D], f32,
                              kind="ExternalOutput")
    gene_bf = (nc.dram_tensor("gene_bf", [N_GENES, D], mybir.dt.bfloat16)
               if EDGE_BF16 else None)

    GC = 7   # cell windows per self/out staging group (98 = 14*7)
    GG = 5   # gene windows per group (20 = 4*5)
    assert WC % GC == 0 and WG % GG == 0

    with tile.TileContext(nc) as tc:
        with (
            tc.tile_pool(name="const", bufs=1) as cpool,
            tc.tile_pool(name="work", bufs=3) as wpool,
            tc.tile_pool(name="small", bufs=3) as spool,
            tc.tile_pool(name="stage", bufs=2) as stpool,
            tc.tile_pool(name="psum", bufs=2, space="PSUM") as ppool,
        ):
            # constants
            w_cs = cpool.tile([D, D], f32, tag="wcs")
            w_cn = cpool.tile([D, D], f32, tag="wcn")
            w_gs = cpool.tile([D, D], f32, tag="wgs")
            w_gn = cpool.tile([D, D], f32, tag="wgn")
            bc_t = cpool.tile([1, D], f32, tag="bc")
            bg_t = cpool.tile([1, D], f32, tag="bg")
            ones1 = cpool.tile([1, D], f32, tag="ones")
            ident = cpool.tile([P, P], f32, tag="ident")
            iota_i = cpool.tile([P, P], i32, tag="iotai")
            iota_f = cpool.tile([P, P], e_dt, tag="iotaf")
            nc.sync.dma_start(w_cs[:], Wcs[:])
            nc.sync.dma_start(w_cn[:], Wcn[:])
            nc.sync.dma_start(w_gs[:], Wgs[:])
            nc.sync.dma_start(w_gn[:], Wgn[:])
            nc.sync.dma_start(bc_t[:], bc[:])
            nc.sync.dma_start(bg_t[:], bg[:])
            nc.vector.memset(ones1[:], 1.0)
            make_identity(nc, ident[:])
            nc.gpsimd.iota(iota_i[:], pattern=[[1, P]], base=0,
                           channel_multiplier=0)
            nc.vector.tensor_copy(iota_f[:], iota_i[:])

            # schedule metadata buffers
            ci_t = cpool.tile([P, 8 * sTC], i16, tag="ci")
            cd_t = cpool.tile([P, sTC], f32, tag="cd")
            cv_t = cpool.tile([P, sTC], f32, tag="cv")
            gi_t = cpool.tile([P, 8 * sTG], i16, tag="gi")
            gd_t = cpool.tile([P, sTG], f32, tag="gd")
            gv_t = cpool.tile([P, sTG], f32, tag="gv")
            nc.sync.dma_start(ci_t[:], c_idx[:])
            nc.sync.dma_start(cd_t[:], c_dest[:])
            nc.sync.dma_start(cv_t[:], c_val[:])
            nc.sync.dma_start(gi_t[:], g_idx[:])
            nc.sync.dma_start(gd_t[:], g_dest[:])
            nc.sync.dma_start(gv_t[:], g_val[:])

            # convert gene_x (f32) -> gene_bf (bf16) in DRAM, in groups;
            # overlaps with the gene side below which doesn't read it
            if EDGE_BF16:
                r = 0
                while r < N_GENES:
                    rows = min(8 * P, N_GENES - r)
                    g = rows // P
                    pp = rows - g * P  # partial tail rows
                    for nb_rows, nb_g in (((g * P), g), (pp, 1)):
                        if nb_rows == 0:
                            continue
                        p_used = min(nb_rows, P)
                        cf = stpool.tile([P, 8, D], f32, tag="cvt_f")
                        cb = stpool.tile([P, 8, D], mybir.dt.bfloat16,
                                         tag="cvt_b")
                        src = gene_x[r:r + nb_rows, :].rearrange(
                            "(g p) f -> p g f", p=p_used)
                        dst = gene_bf[r:r + nb_rows, :].rearrange(
                            "(g p) f -> p g f", p=p_used)
                        nc.sync.dma_start(cf[:p_used, :nb_g, :], src)
                        nc.vector.tensor_copy(cb[:p_used, :nb_g, :],
                                              cf[:p_used, :nb_g, :])
                        nc.sync.dma_start(dst, cb[:p_used, :nb_g, :])
                        r += nb_rows

            def side(W, G, sched, table, table_rows, self_dram, out_dram,
                     idx_t, dest_t, val_t, w_self, w_neigh, bias_t, name,
                     table_is_bf):
                col_i = 0
                col_f = 0
                for g0 in range(0, W, G):
                    st_self = stpool.tile([P, G, D], f32, tag=f"{name}_self")
                    st_out = stpool.tile([P, G, D], f32, tag=f"{name}_out")
                    nc.sync.dma_start(
                        st_self[:],
                        self_dram[g0 * P:(g0 + G) * P, :].rearrange(
                            "(g p) f -> p g f", p=P),
                    )
                    for wg in range(G):
                        w = g0 + wg
                        # ---- neighbor segment-sum into PSUM ----
                        nb = ppool.tile([P, D], f32, tag="nb", space="PSUM")
                        groups = sched[w]
                        total_t = sum(t for _, t in groups)
                        ti = 0
                        for k, t_full in groups:
                          k0 = k * CHUNK
                          k1 = min(k0 + CHUNK, table_rows)
                          # dma_gather crashes above num_idxs=1024: split calls
                          for t0 in range(0, t_full, 8):
                            t = min(8, t_full - t0)
                            gdt = e_dt if table_is_bf else f32
                            if "gather" in ablate:
                                msg = None
                            else:
                                msg = wpool.tile([P, t, D], gdt, tag="msg")
                                nc.gpsimd.dma_gather(
                                    out_ap=msg[:],
                                    in_ap=table[k0:k1, :],
                                    idxs_ap=idx_t[:, col_i:col_i + 8 * t],
                                    num_idxs=t * P,
                                    num_idxs_reg=t * P,
                                    elem_size=D,
                                )
                            col_i += 8 * t
                            if (EDGE_BF16 and not table_is_bf
                                    and msg is not None
                                    and "cvt" not in ablate):
                                msgb = wpool.tile([P, t, D], e_dt, tag="msgb")
                                nc.vector.tensor_copy(msgb[:], msg[:])
                                msg = msgb
                            for tt in range(t):
                                oh = wpool.tile([P, P], e_dt, tag="oh")
                                if "oh" not in ablate:
                                    nc.vector.tensor_scalar(
                                        oh[:], iota_f[:],
                                        dest_t[:, col_f:col_f + 1],
                                        val_t[:, col_f:col_f + 1],
                                        import_mybir().AluOpType.is_equal,
                                        import_mybir().AluOpType.mult,
                                    )
                                if "mm" not in ablate:
                                    # nb_T[f, c] += msg.T @ onehot, i.e. the
                                    # neighbor sum accumulated pre-transposed
                                    lh = (iota_f[:] if msg is None
                                          else msg[:, tt, :])
                                    rh = iota_f[:] if "oh" in ablate else oh[:]
                                    nc.tensor.matmul(
                                        nb[:], lhsT=lh, rhs=rh,
                                        start=(ti == 0),
                                        stop=(ti == total_t - 1),
                                        skip_group_check=True,
                                    )
                                col_f += 1
                                ti += 1
                        if "mm" in ablate:
                            nc.tensor.matmul(nb[:], lhsT=iota_f[:],
                                             rhs=iota_f[:], start=True,
                                             stop=True, skip_group_check=True)
                        # ---- combine: nb is already [f_in, c]; the self
                        # tile arrives host-pre-transposed as [f_in, c] ----
                        nbT = spool.tile([P, P], f32, tag="nbT")
                        nc.vector.tensor_copy(nbT[:], nb[:])

                        acc = ppool.tile([P, D], f32, tag="acc", space="PSUM")
                        nc.tensor.matmul(acc[:], lhsT=st_self[:, wg, :],
                                         rhs=w_self[:],
                                         start=True, stop=False,
                                         skip_group_check=True)
                        nc.tensor.matmul(acc[:], lhsT=ones1[:], rhs=bias_t[:],
                                         start=False, stop=False,
                                         skip_group_check=True)
                        nc.tensor.matmul(acc[:], lhsT=nbT[:], rhs=w_neigh[:],
                                         start=False, stop=True,
                                         skip_group_check=True)
                        nc.scalar.activation(
                            st_out[:, wg, :], acc[:],
                            import_mybir().ActivationFunctionType.Relu,
                        )
                    nc.sync.dma_start(
                        out_dram[g0 * P:(g0 + G) * P, :].rearrange(
                            "(g p) f -> p g f", p=P),
                        st_out[:],
                    )

            # gene side first: its gathers read cell_x (f32) and overlap the
            # gene_x->bf16 conversion; cell side then gathers from gene_bf
            side(WG, GG, meta_g["sched"], cell_x, N_CELLS, gene_self,
                 gene_out, gi_t, gd_t, gv_t, w_gs, w_gn, bg_t, "g", False)
            side(WC, GC, meta_c["sched"],
                 gene_bf if EDGE_BF16 else gene_x, N_GENES, cell_self,
                 cell_out, ci_t, cd_t, cv_t, w_cs, w_cn, bc_t, "c",
                 EDGE_BF16)

    nc.compile()
    return nc


def import_mybir():
    import concourse.mybir as mybir
    return mybir


def _prepare(inputs):
    cell_x = np.asarray(inputs["cell_x"], dtype=np.float32)
    gene_x = np.asarray(inputs["gene_x"], dtype=np.float32)
    edge_row = np.asarray(inputs["edge_row"], dtype=np.int64)
    edge_col = np.asarray(inputs["edge_col"], dtype=np.int64)
    edge_val = np.asarray(inputs["edge_val"], dtype=np.float32)

    # cell side: dest=cells (edge_row), src=genes (edge_col), table=gene_x
    meta_c, pc_c = _prep_side(edge_row, edge_col, edge_val, N_CELLS, N_GENES)
    # gene side: dest=genes (edge_col), src=cells (edge_row), table=cell_x
    meta_g, pc_g = _prep_side(edge_col, edge_row, edge_val, N_GENES, N_CELLS)

    bc = (np.asarray(inputs["b_cs"]) + np.asarray(inputs["b_cn"])).astype(
        np.float32).reshape(1, D)
    bg = (np.asarray(inputs["b_gs"]) + np.asarray(inputs["b_gn"])).astype(
        np.float32).reshape(1, D)

    in_maps = []
    for c in range(NCORES):
        in_maps.append({
            "gene_x": gene_x,
            "cell_x": cell_x,
            "cell_self": _self_slice(cell_x, int(meta_c["starts"][c]),
                                     int(meta_c["counts"][c]), meta_c["W"]),
            "gene_self": _self_slice(gene_x, int(meta_g["starts"][c]),
                                     int(meta_g["counts"][c]), meta_g["W"]),
            "c_idx": pc_c[c]["idx"],
            "c_dest": pc_c[c]["dest"],
            "c_val": pc_c[c]["val"],
            "g_idx": pc_g[c]["idx"],
            "g_dest": pc_g[c]["dest"],
            "g_val": pc_g[c]["val"],
            "Wcs": np.ascontiguousarray(inputs["W_cs"], dtype=np.float32),
            "Wcn": np.ascontiguousarray(inputs["W_cn"], dtype=np.float32),
            "Wgs": np.ascontiguousarray(inputs["W_gs"], dtype=np.float32),
            "Wgn": np.ascontiguousarray(inputs["W_gn"], dtype=np.float32),
            "bc": bc,
            "bg": bg,
        })
    return meta_c, meta_g, in_maps


def _merge(meta, outs, key, n_rows):
    full = np.zeros((n_rows, D), dtype=np.float32)
    for c in range(NCORES):
        a = int(meta["starts"][c]) * P
        n = int(meta["counts"][c]) * P
        e = min(a + n, n_rows)
        full[a:e] = outs[c][key][: e - a]
    return full


def kernel(**inputs):
    global LAST_RESULT
    from concourse.bass_utils import run_bass_kernel_spmd

    meta_c, meta_g, in_maps = _prepare(inputs)
    nc = _build_nc(meta_c, meta_g)
    res = run_bass_kernel_spmd(nc, in_maps, core_ids=list(range(NCORES)),
                               trace=TRACE)
    LAST_RESULT = res
    cell_out = _merge(meta_c, res.results, "cell_out_part", N_CELLS)
    gene_out = _merge(meta_g, res.results, "gene_out_part", N_GENES)
    return cell_out, gene_out


# revision 37
# speedup vs baseline: 154.4535x; 154.4535x over previous
"""Bipartite graph convolution (cell/gene GNN message passing) on 8 Trainium2
NeuronCores.

Strategy: shard by *destination* node blocks (128 nodes per block). Each core
owns a contiguous range of cell blocks and gene blocks, so every edge's
segment-sum contribution lands on exactly one core -- no cross-core reduction
or collectives at all. Per destination window (128 nodes) the core:
  1. dma_gather's the source-node feature rows for all of the window's edges
     (packed into large multi-window calls, single_packet=False; indices are
     int16 so >32k source tables are gathered per 32768-row chunk),
  2. builds a weighted one-hot [edge, dest_local] fp16 matrix with a single
     DVE tensor_scalar: (iota == dest) * val,
  3. accumulates nb[f, c] += msg.T @ onehot into PSUM on the TensorEngine --
     the segment sum, produced PRE-TRANSPOSED so it feeds the neighbor
     weight matmul as lhsT with no PE transpose,
  4. computes relu(x_selfT.T @ W_s + outer(1, bias) + nb.T @ W_n) in one PSUM
     accumulation group (self tiles arrive host-pre-transposed), ReLU on the
     ScalarEngine, staged group stores.
Edge messages/one-hots/self/weights run in fp16 (fp32 matmul is 4x slower on
the PE; fp16 keeps ~3e-4 relative error), accumulation is always fp32 in
PSUM. The gene_x table is converted to an fp16 DRAM copy on-device, hidden
under the gene-side phase which gathers from the f32 cell_x table directly.
Host work is index bucketing/padding (the sharding) and output reassembly.
"""

import sys

sys.path.insert(0, "/opt/trn_rl_repo")

import numpy as np

N_CELLS = 100000
N_GENES = 20000
N_EDGES = 640000
D = 128
P = 128
NCORES = 8
CHUNK = 32768  # int16 index range for dma_gather

# knobs / debug state
TRACE = False
LAST_RESULT = None
LAST_NC = None
EDGE_BF16 = True  # fp16 edge messages/onehots (4x faster PE, ~2e-4 rel err)
TUNE = {}  # optional overrides: GC, GG, MC, WB (work), PB (psum), SB (stage)


def _ceil_div(a, b):
    return (a + b - 1) // b


def _core_block_ranges(n_blocks):
    """Split n_blocks into NCORES contiguous ranges as evenly as possible.
    Returns (starts, counts)."""
    base = n_blocks // NCORES
    rem = n_blocks % NCORES
    counts = np.array([base + (1 if c < rem else 0) for c in range(NCORES)])
    starts = np.concatenate([[0], np.cumsum(counts)[:-1]])
    return starts, counts


def _wrap_idx_i16(idx):
    """dma_gather index layout: index i lives at partition i%16, column i//16,
    replicated across the 8 partition groups of 16."""
    n = idx.shape[0]
    assert n % 16 == 0
    block16 = idx.reshape(n // 16, 16).T.astype(np.int16)  # [16, n/16]
    return np.tile(block16, (8, 1))  # [128, n/16]


def _prep_side(dest, src, val, n_dest, n_src):
    """Bucket edges by destination block (and source chunk), build the uniform
    per-core schedule and per-core device input arrays.

    Returns dict with:
      W: window slots per core
      sched: list of (k_chunk, T) lists per window (same for all cores)
      starts, counts: block ranges per core
      per_core: list of dicts with idx_i16 [128, 8*sumT], dest_f32 [128, sumT],
                val_f32 [128, sumT]
    """
    n_blocks = _ceil_div(n_dest, P)
    n_chunks = _ceil_div(n_src, CHUNK)
    starts, counts = _core_block_ranges(n_blocks)
    W = int(counts.max())

    block = dest // P
    chunk = src // CHUNK
    # sort edges by (block, chunk)
    order = np.lexsort((chunk, block))
    d_s = dest[order]
    s_s = src[order]
    v_s = val[order]
    key = block[order] * n_chunks + chunk[order]
    cnt = np.bincount(key, minlength=n_blocks * n_chunks).reshape(
        n_blocks, n_chunks
    )
    seg_off = np.concatenate([[0], np.cumsum(cnt.ravel())])

    # schedule: T[w][k] = max over cores of tiles needed
    T = np.zeros((W, n_chunks), dtype=np.int64)
    for c in range(NCORES):
        for w in range(int(counts[c])):
            b = int(starts[c]) + w
            T[w] = np.maximum(T[w], _ceil_div(cnt[b], P))
    # ensure at least one tile per window so every window has a defined
    # neighbor accumulation (zeros)
    for w in range(W):
        if T[w].sum() == 0:
            T[w][0] = 1

    sched = [[(k, int(T[w][k])) for k in range(n_chunks) if T[w][k] > 0]
             for w in range(W)]
    sum_T = int(T.sum())

    per_core = []
    for c in range(NCORES):
        idx_cols = []
        dest_cols = []
        val_cols = []
        for w in range(W):
            b = int(starts[c]) + w
            real = w < int(counts[c])
            for k, t in sched[w]:
                n_slots = t * P
                idx = np.zeros(n_slots, dtype=np.int64)
                dst = np.full(n_slots, -1.0, dtype=np.float32)
                vv = np.zeros(n_slots, dtype=np.float32)
                if real:
                    a = seg_off[b * n_chunks + k]
                    e = seg_off[b * n_chunks + k + 1]
                    m = e - a
                    assert m <= n_slots
                    if m > 0:
                        idx[:m] = s_s[a:e] - k * CHUNK
                        dst[:m] = (d_s[a:e] - b * P).astype(np.float32)
                        vv[:m] = v_s[a:e]
                idx_cols.append(_wrap_idx_i16(idx))
                dest_cols.append(dst.reshape(t, P).T)
                val_cols.append(vv.reshape(t, P).T)
        per_core.append({
            "idx": np.concatenate(idx_cols, axis=1),
            "dest": np.ascontiguousarray(np.concatenate(dest_cols, axis=1)),
            "val": np.ascontiguousarray(np.concatenate(val_cols, axis=1)),
        })
    return {
        "W": W,
        "sched": sched,
        "starts": starts,
        "counts": counts,
        "sum_T": sum_T,
        "n_chunks": n_chunks,
    }, per_core


def _self_slice(x, start_block, n_blocks_core, W):
    """Compact per-core self-feature tensor, BLOCK-TRANSPOSED: row (w*P + f)
    holds feature f of the window's 128 nodes, so the SBUF staging tile is
    directly usable as matmul lhsT [f_in, c]. Zero-padded."""
    out = np.zeros((W, P, D), dtype=np.float32)
    a = start_block * P
    e = min(a + n_blocks_core * P, x.shape[0])
    out.reshape(W * P, D)[: e - a] = x[a:e]
    dt = np.float16 if EDGE_BF16 else np.float32
    return np.ascontiguousarray(out.transpose(0, 2, 1)).reshape(
        W * P, D).astype(dt)


def _build_nc(meta_c, meta_g, ablate=()):
    import concourse.mybir as mybir
    import concourse.tile as tile
    from concourse import bacc

    f32 = mybir.dt.float32
    bf16 = mybir.dt.float16
    i16 = mybir.dt.int16
    i32 = mybir.dt.int32
    e_dt = bf16 if EDGE_BF16 else f32
    WC, WG = meta_c["W"], meta_g["W"]
    sTC, sTG = meta_c["sum_T"], meta_g["sum_T"]

    nc = bacc.Bacc("TRN2", target_bir_lowering=False, debug=False,
                   num_devices=NCORES)

    # DRAM tensors
    gene_x = nc.dram_tensor("gene_x", [N_GENES, D], f32, kind="ExternalInput")
    cell_x = nc.dram_tensor("cell_x", [N_CELLS, D], f32, kind="ExternalInput")
    cell_self = nc.dram_tensor("cell_self", [WC * P, D], e_dt,
                               kind="ExternalInput")
    gene_self = nc.dram_tensor("gene_self", [WG * P, D], e_dt,
                               kind="ExternalInput")
    c_idx = nc.dram_tensor("c_idx", [P, 8 * sTC], i16, kind="ExternalInput")
    c_dest = nc.dram_tensor("c_dest", [P, sTC], f32, kind="ExternalInput")
    c_val = nc.dram_tensor("c_val", [P, sTC], f32, kind="ExternalInput")
    g_idx = nc.dram_tensor("g_idx", [P, 8 * sTG], i16, kind="ExternalInput")
    g_dest = nc.dram_tensor("g_dest", [P, sTG], f32, kind="ExternalInput")
    g_val = nc.dram_tensor("g_val", [P, sTG], f32, kind="ExternalInput")
    Wcs = nc.dram_tensor("Wcs", [D, D], e_dt, kind="ExternalInput")
    Wcn = nc.dram_tensor("Wcn", [D, D], e_dt, kind="ExternalInput")
    Wgs = nc.dram_tensor("Wgs", [D, D], e_dt, kind="ExternalInput")
    Wgn = nc.dram_tensor("Wgn", [D, D], e_dt, kind="ExternalInput")
    bc = nc.dram_tensor("bc", [1, D], e_dt, kind="ExternalInput")
    bg = nc.dram_tensor("bg", [1, D], e_dt, kind="ExternalInput")
    cell_out = nc.dram_tensor("cell_out_part", [WC * P, D], f32,
                              kind="ExternalOutput")
    gene_out = nc.dram_tensor("gene_out_part", [WG * P, D], f32,
                              kind="ExternalOutput")
    gene_bf = (nc.dram_tensor("gene_bf", [N_GENES, D], mybir.dt.float16)
               if EDGE_BF16 else None)

    GC = TUNE.get("GC", 7)   # cell windows per self/out staging group
    GG = TUNE.get("GG", 5)   # gene windows per group
    assert WC % GC == 0 and WG % GG == 0

    with tile.TileContext(nc) as tc:
        with (
            tc.tile_pool(name="const", bufs=1) as cpool,
            tc.tile_pool(name="work", bufs=TUNE.get("WB", 6)) as wpool,
            tc.tile_pool(name="small", bufs=3) as spool,
            tc.tile_pool(name="stage", bufs=TUNE.get("SB", 2)) as stpool,
            tc.tile_pool(name="psum", bufs=TUNE.get("PB", 4), space="PSUM") as ppool,
        ):
            # constants
            w_cs = cpool.tile([D, D], e_dt, tag="wcs")
            w_cn = cpool.tile([D, D], e_dt, tag="wcn")
            w_gs = cpool.tile([D, D], e_dt, tag="wgs")
            w_gn = cpool.tile([D, D], e_dt, tag="wgn")
            bc_t = cpool.tile([1, D], e_dt, tag="bc")
            bg_t = cpool.tile([1, D], e_dt, tag="bg")
            ones1 = cpool.tile([1, D], e_dt, tag="ones")
            iota_i = cpool.tile([P, P], i32, tag="iotai")
            iota_f = cpool.tile([P, P], e_dt, tag="iotaf")
            nc.sync.dma_start(w_cs[:], Wcs[:])
            nc.sync.dma_start(w_cn[:], Wcn[:])
            nc.sync.dma_start(w_gs[:], Wgs[:])
            nc.sync.dma_start(w_gn[:], Wgn[:])
            nc.sync.dma_start(bc_t[:], bc[:])
            nc.sync.dma_start(bg_t[:], bg[:])
            nc.vector.memset(ones1[:], 1.0)
            nc.gpsimd.iota(iota_i[:], pattern=[[1, P]], base=0,
                           channel_multiplier=0)
            nc.vector.tensor_copy(iota_f[:], iota_i[:])

            # schedule metadata buffers
            ci_t = cpool.tile([P, 8 * sTC], i16, tag="ci")
            cd_t = cpool.tile([P, sTC], f32, tag="cd")
            cv_t = cpool.tile([P, sTC], f32, tag="cv")
            gi_t = cpool.tile([P, 8 * sTG], i16, tag="gi")
            gd_t = cpool.tile([P, sTG], f32, tag="gd")
            gv_t = cpool.tile([P, sTG], f32, tag="gv")
            # gene side runs first: load its metadata first, idx split so
            # the first gathers aren't gated on the full buffer transfer
            h = (8 * sTG) // 2
            nc.sync.dma_start(gi_t[:, :h], g_idx[:, :h])
            nc.sync.dma_start(gd_t[:], g_dest[:])
            nc.sync.dma_start(gv_t[:], g_val[:])
            nc.sync.dma_start(gi_t[:, h:], g_idx[:, h:])
            nc.sync.dma_start(ci_t[:], c_idx[:])
            nc.sync.dma_start(cd_t[:], c_dest[:])
            nc.sync.dma_start(cv_t[:], c_val[:])

            # convert gene_x (f32) -> gene_bf (bf16) in DRAM, in groups;
            # overlaps with the gene side below which doesn't read it
            if EDGE_BF16:
                r = 0
                while r < N_GENES:
                    rows = min(8 * P, N_GENES - r)
                    g = rows // P
                    pp = rows - g * P  # partial tail rows
                    for nb_rows, nb_g in (((g * P), g), (pp, 1)):
                        if nb_rows == 0:
                            continue
                        p_used = min(nb_rows, P)
                        cf = stpool.tile([P, 8, D], f32, tag="cvt_f")
                        cb = stpool.tile([P, 8, D], mybir.dt.float16,
                                         tag="cvt_b")
                        src = gene_x[r:r + nb_rows, :].rearrange(
                            "(g p) f -> p g f", p=p_used)
                        dst = gene_bf[r:r + nb_rows, :].rearrange(
                            "(g p) f -> p g f", p=p_used)
                        nc.sync.dma_start(cf[:p_used, :nb_g, :], src)
                        nc.vector.tensor_copy(cb[:p_used, :nb_g, :],
                                              cf[:p_used, :nb_g, :])
                        nc.sync.dma_start(dst, cb[:p_used, :nb_g, :])
                        r += nb_rows

            def side(W, G, sched, table, table_rows, self_dram, out_dram,
                     idx_t, dest_t, val_t, w_self, w_neigh, bias_t, name,
                     table_is_bf):
                gdt = e_dt if table_is_bf else f32
                n_chunks = max(k for w in range(W) for k, _ in sched[w]) + 1

                # ---- gather call plan: pack tiles into large dma_gather
                # calls (single_packet=False lifts the 1024-idx cap). Calls
                # must stay within one source chunk; with a single chunk we
                # pack a whole staging group (~G windows) per call. ----
                call_plan = []  # list of n_tiles per call, in stream order
                if n_chunks == 1:
                    for g0 in range(0, W, G):
                        n = sum(t for w in range(g0, min(g0 + G, W))
                                for _, t in sched[w])
                        call_plan.append((0, n))
                else:
                    for w in range(W):
                        for k, t in sched[w]:
                            call_plan.append((k, t))
                call_plan = [c for c in call_plan if c[1] > 0]
                mc = TUNE.get("MC", 12)  # max tiles per gather call (0 = off)
                if mc:
                    split = []
                    for k, n in call_plan:
                        while n > 0:
                            split.append((k, min(mc, n)))
                            n -= mc
                    call_plan = split

                state = {"call": 0, "msg": None, "off": 0, "n": 0,
                         "col_i": 0}

                def next_tile():
                    if "gather" in ablate:
                        return None
                    if state["off"] == state["n"]:
                        k, n = call_plan[state["call"]]
                        state["call"] += 1
                        k0 = k * CHUNK
                        k1 = min(k0 + CHUNK, table_rows)
                        msg = wpool.tile([P, n, D], gdt, tag="msg")
                        nc.gpsimd.dma_gather(
                            out_ap=msg[:],
                            in_ap=table[k0:k1, :],
                            idxs_ap=idx_t[:, state["col_i"]:
                                          state["col_i"] + 8 * n],
                            num_idxs=n * P,
                            num_idxs_reg=n * P,
                            elem_size=D,
                            single_packet=False,
                        )
                        state["col_i"] += 8 * n
                        if EDGE_BF16 and not table_is_bf:
                            msgb = wpool.tile([P, n, D], e_dt, tag="msgb")
                            nc.vector.tensor_copy(msgb[:], msg[:])
                            msg = msgb
                        state["msg"] = msg
                        state["off"] = 0
                        state["n"] = n
                    sl = state["msg"][:, state["off"], :]
                    state["off"] += 1
                    return sl

                col_f = 0
                for g0 in range(0, W, G):
                    st_self = stpool.tile([P, G, D], e_dt, tag=f"{name}_self")
                    st_out = stpool.tile([P, G, D], f32, tag=f"{name}_out")
                    nc.sync.dma_start(
                        st_self[:],
                        self_dram[g0 * P:(g0 + G) * P, :].rearrange(
                            "(g p) f -> p g f", p=P),
                    )
                    for wg in range(G):
                        w = g0 + wg
                        # ---- neighbor segment-sum into PSUM ----
                        nb = ppool.tile([P, D], f32, tag="nb", space="PSUM")
                        groups = sched[w]
                        total_t = sum(t for _, t in groups)
                        ti = 0
                        for k, t_full in groups:
                            for tt in range(t_full):
                                msg_sl = next_tile()
                                oh = wpool.tile([P, P], e_dt, tag="oh")
                                if "oh" not in ablate:
                                    nc.vector.tensor_scalar(
                                        oh[:], iota_f[:],
                                        dest_t[:, col_f:col_f + 1],
                                        val_t[:, col_f:col_f + 1],
                                        import_mybir().AluOpType.is_equal,
                                        import_mybir().AluOpType.mult,
                                    )
                                if "mm" not in ablate:
                                    # nb_T[f, c] += msg.T @ onehot, i.e. the
                                    # neighbor sum accumulated pre-transposed
                                    lh = (iota_f[:] if "gather" in ablate
                                          else msg_sl)
                                    rh = iota_f[:] if "oh" in ablate else oh[:]
                                    nc.tensor.matmul(
                                        nb[:], lhsT=lh, rhs=rh,
                                        start=(ti == 0),
                                        stop=(ti == total_t - 1),
                                        skip_group_check=True,
                                    )
                                col_f += 1
                                ti += 1
                        if "mm" in ablate:
                            nc.tensor.matmul(nb[:], lhsT=iota_f[:],
                                             rhs=iota_f[:], start=True,
                                             stop=True, skip_group_check=True)
                        # ---- combine: nb is already [f_in, c]; the self
                        # tile arrives host-pre-transposed as [f_in, c] ----
                        if "epi" in ablate:
                            nc.scalar.activation(
                                st_out[:, wg, :], nb[:],
                                import_mybir().ActivationFunctionType.Relu,
                            )
                        else:
                            nbT = spool.tile([P, P], e_dt, tag="nbT")
                            nc.vector.tensor_copy(nbT[:], nb[:])

                            acc = ppool.tile([P, D], f32, tag="acc",
                                             space="PSUM")
                            nc.tensor.matmul(acc[:], lhsT=st_self[:, wg, :],
                                             rhs=w_self[:],
                                             start=True, stop=False,
                                             skip_group_check=True)
                            nc.tensor.matmul(acc[:], lhsT=ones1[:],
                                             rhs=bias_t[:],
                                             start=False, stop=False,
                                             skip_group_check=True)
                            nc.tensor.matmul(acc[:], lhsT=nbT[:],
                                             rhs=w_neigh[:],
                                             start=False, stop=True,
                                             skip_group_check=True)
                            nc.scalar.activation(
                                st_out[:, wg, :], acc[:],
                                import_mybir().ActivationFunctionType.Relu,
                            )
                    nc.sync.dma_start(
                        out_dram[g0 * P:(g0 + G) * P, :].rearrange(
                            "(g p) f -> p g f", p=P),
                        st_out[:],
                    )

            # gene side first: its gathers read cell_x (f32) and overlap the
            # gene_x->bf16 conversion; cell side then gathers from gene_bf
            side(WG, GG, meta_g["sched"], cell_x, N_CELLS, gene_self,
                 gene_out, gi_t, gd_t, gv_t, w_gs, w_gn, bg_t, "g", False)
            side(WC, GC, meta_c["sched"],
                 gene_bf if EDGE_BF16 else gene_x, N_GENES, cell_self,
                 cell_out, ci_t, cd_t, cv_t, w_cs, w_cn, bc_t, "c",
                 EDGE_BF16)

    nc.compile()
    return nc


def import_mybir():
    import concourse.mybir as mybir
    return mybir


def _prepare(inputs):
    cell_x = np.asarray(inputs["cell_x"], dtype=np.float32)
    gene_x = np.asarray(inputs["gene_x"], dtype=np.float32)
    edge_row = np.asarray(inputs["edge_row"], dtype=np.int64)
    edge_col = np.asarray(inputs["edge_col"], dtype=np.int64)
    edge_val = np.asarray(inputs["edge_val"], dtype=np.float32)

    # cell side: dest=cells (edge_row), src=genes (edge_col), table=gene_x
    meta_c, pc_c = _prep_side(edge_row, edge_col, edge_val, N_CELLS, N_GENES)
    # gene side: dest=genes (edge_col), src=cells (edge_row), table=cell_x
    meta_g, pc_g = _prep_side(edge_col, edge_row, edge_val, N_GENES, N_CELLS)

    _edt = np.float16 if EDGE_BF16 else np.float32
    bc = (np.asarray(inputs["b_cs"]) + np.asarray(inputs["b_cn"])).astype(
        _edt).reshape(1, D)
    bg = (np.asarray(inputs["b_gs"]) + np.asarray(inputs["b_gn"])).astype(
        _edt).reshape(1, D)

    in_maps = []
    for c in range(NCORES):
        in_maps.append({
            "gene_x": gene_x,
            "cell_x": cell_x,
            "cell_self": _self_slice(cell_x, int(meta_c["starts"][c]),
                                     int(meta_c["counts"][c]), meta_c["W"]),
            "gene_self": _self_slice(gene_x, int(meta_g["starts"][c]),
                                     int(meta_g["counts"][c]), meta_g["W"]),
            "c_idx": pc_c[c]["idx"],
            "c_dest": pc_c[c]["dest"],
            "c_val": pc_c[c]["val"],
            "g_idx": pc_g[c]["idx"],
            "g_dest": pc_g[c]["dest"],
            "g_val": pc_g[c]["val"],
            "Wcs": np.ascontiguousarray(inputs["W_cs"], dtype=_edt),
            "Wcn": np.ascontiguousarray(inputs["W_cn"], dtype=_edt),
            "Wgs": np.ascontiguousarray(inputs["W_gs"], dtype=_edt),
            "Wgn": np.ascontiguousarray(inputs["W_gn"], dtype=_edt),
            "bc": bc,
            "bg": bg,
        })
    return meta_c, meta_g, in_maps


def _merge(meta, outs, key, n_rows):
    full = np.zeros((n_rows, D), dtype=np.float32)
    for c in range(NCORES):
        a = int(meta["starts"][c]) * P
        n = int(meta["counts"][c]) * P
        e = min(a + n, n_rows)
        full[a:e] = outs[c][key][: e - a]
    return full


def kernel(**inputs):
    global LAST_RESULT, LAST_NC
    from concourse.bass_utils import run_bass_kernel_spmd

    meta_c, meta_g, in_maps = _prepare(inputs)
    nc = _build_nc(meta_c, meta_g)
    LAST_NC = nc
    res = run_bass_kernel_spmd(nc, in_maps, core_ids=list(range(NCORES)),
                               trace=TRACE)
    LAST_RESULT = res
    cell_out = _merge(meta_c, res.results, "cell_out_part", N_CELLS)
    gene_out = _merge(meta_g, res.results, "gene_out_part", N_GENES)
    return cell_out, gene_out

